# revision 1
# baseline (speedup 1.0000x reference)
"""Causal self-attention (token-shift + QK-RMSNorm + RoPE + value-residual)
Trainium2 Bass kernel, sharded over 8 NeuronCores.

Sharding: core c handles batch b=c//4 and head-group g=c%4 (4 heads, 512
channels). Each core computes q/k/v projections for its channels, attention
for its heads, and a partial c_proj (its 512 input rows of Wproj). Host sums
the 4 partials per batch and adds the residual.

All matmuls are bf16 x bf16 with fp32 PSUM accumulation. Device layout is
transposed ([channel, time]) so the contraction dim always sits on SBUF
partitions; scores are computed as S^T = K^T_tile.T-style matmuls, softmax
uses a ones-matmul partition-broadcast row-sum (no max subtraction needed:
|scores| <= sqrt(D) after RMS norm), and RoPE's half-swap goes through a
SBUF->SBUF DMA (elementwise engines cannot cross partition bases).
"""
import sys

sys.path.insert(0, "/opt/trn_rl_repo")

import numpy as np
import ml_dtypes

B, T, C, H, D = 2, 2048, 2048, 16, 128
NCORES = 8
LC = 512          # local channels per core (4 heads)
TQ = 512          # tq chunk size
NKT = C // 128    # 16 k-tiles over the C contraction
NCHUNK = T // TQ  # 4
ROPE_THETA = 10000.0
MASK_NEG = -1.0e5
EPS = float(np.finfo(np.float32).eps)

_bf = ml_dtypes.bfloat16

_prog_cache = {}


def _build_program():
    import concourse.bass as bass
    import concourse.mybir as mybir
    from concourse import bacc
    from concourse.tile import TileContext
    from concourse.alu_op_type import AluOpType

    AFt = mybir.ActivationFunctionType
    if not getattr(bacc, "_act_tables_pinned", False):
        _orig_gat = bacc.get_activation_tables

        def _pinned_gat(arch):
            tables = _orig_gat(arch)
            pinned = {AFt.Ln, AFt.Exp, AFt.Square}
            for name, fns in tables.items():
                if name != "natural_log_exp_and_others":
                    fns -= pinned
            return tables

        bacc.get_activation_tables = _pinned_gat
        bacc._act_tables_pinned = True

    F32 = mybir.dt.float32
    BF16 = mybir.dt.bfloat16
    AF = mybir.ActivationFunctionType

    nc = bacc.Bacc("TRN2", target_bir_lowering=False, debug=False)

    xbT = nc.dram_tensor("xbT", [C, T], BF16, kind="ExternalInput").ap()
    wq = nc.dram_tensor("wq", [C, LC], BF16, kind="ExternalInput").ap()
    wk = nc.dram_tensor("wk", [C, LC], BF16, kind="ExternalInput").ap()
    wv = nc.dram_tensor("wv", [C, LC], BF16, kind="ExternalInput").ap()
    wp = nc.dram_tensor("wp", [LC, C], BF16, kind="ExternalInput").ap()
    v1l = nc.dram_tensor("v1l", [T, LC], BF16, kind="ExternalInput").ap()
    cos2 = nc.dram_tensor("cos2", [128, T], BF16, kind="ExternalInput").ap()
    sin2 = nc.dram_tensor("sin2", [128, T], BF16, kind="ExternalInput").ap()
    masks = nc.dram_tensor("masks", [128, 896], F32, kind="ExternalInput").ap()
    perm = nc.dram_tensor("perm", [128, 128], BF16, kind="ExternalInput").ap()
    outT = nc.dram_tensor("outT", [C, T], F32, kind="ExternalOutput").ap()

    SCALE = 1.0 / float(np.sqrt(D))

    with TileContext(nc) as tc:
        with (
            tc.tile_pool(name="cpool", bufs=1) as cpool,
            tc.tile_pool(name="kvpool", bufs=1) as kvpool,
            tc.tile_pool(name="xpool", bufs=2) as xpool,
            tc.tile_pool(name="qpool", bufs=2) as qpool,
            tc.tile_pool(name="apool", bufs=2) as apool,
            tc.tile_pool(name="epool", bufs=4) as epool,
            tc.tile_pool(name="wpool", bufs=2) as wpool,
            tc.tile_pool(name="opool", bufs=3) as opool,
            tc.tile_pool(name="pspool", bufs=1, space="PSUM") as pspool,
        ):
            # ---- constant loads (before barrier) ----
            wq_t = []
            wk_t = []
            wv_t = []
            for kt in range(NKT):
                t_ = cpool.tile([128, LC], BF16, tag=f"wq{kt}", name=f"wq{kt}")
                nc.sync.dma_start(t_, wq[128 * kt:128 * kt + 128, :])
                wq_t.append(t_)
                t_ = cpool.tile([128, LC], BF16, tag=f"wk{kt}", name=f"wk{kt}")
                nc.sync.dma_start(t_, wk[128 * kt:128 * kt + 128, :])
                wk_t.append(t_)
                t_ = cpool.tile([128, LC], BF16, tag=f"wv{kt}", name=f"wv{kt}")
                nc.sync.dma_start(t_, wv[128 * kt:128 * kt + 128, :])
                wv_t.append(t_)
            wp_t = []
            for m in range(4):
                t_ = cpool.tile([128, C], BF16, tag=f"wp{m}", name=f"wp{m}")
                nc.sync.dma_start(t_, wp[128 * m:128 * m + 128, :])
                wp_t.append(t_)
            cos_sb = cpool.tile([128, T], BF16, tag="cos", name="cos_sb")
            nc.sync.dma_start(cos_sb, cos2)
            sin_sb = cpool.tile([128, T], BF16, tag="sin", name="sin_sb")
            nc.sync.dma_start(sin_sb, sin2)
            mask_sb = cpool.tile([128, 896], F32, tag="mask", name="mask_sb")
            nc.sync.dma_start(mask_sb, masks)
            ones = cpool.tile([128, 128], BF16, tag="ones", name="ones")
            nc.vector.memset(ones, 1.0)
            epst = cpool.tile([128, 1], F32, tag="epst", name="epst")
            nc.vector.memset(epst, EPS)
            perm_sb = cpool.tile([128, 128], BF16, tag="perm", name="perm_sb")
            nc.sync.dma_start(perm_sb, perm)

            # persistent K^T (per head) and V stores
            kT = [
                kvpool.tile([128, T], BF16, tag=f"kT{m}", name=f"kT{m}")
                for m in range(4)
            ]
            vst = [
                kvpool.tile([128, LC], BF16, tag=f"v{i}", name=f"v{i}")
                for i in range(T // 128)
            ]

            for j in range(NCHUNK):
                tq0 = TQ * j
                # ---- load xb chunk ----
                xb_t = []
                for kt in range(NKT):
                    t_ = xpool.tile([128, TQ], BF16, tag=f"xb{kt}", name=f"xb{kt}_{j}")
                    nc.sync.dma_start(t_, xbT[128 * kt:128 * kt + 128, tq0:tq0 + TQ])
                    xb_t.append(t_)

                # ---- q/k projections + rmsnorm + rope ----
                qT = []
                for which, wt in (("q", wq_t), ("k", wk_t)):
                    for m in range(4):
                        q_ps = pspool.tile([128, TQ], F32, tag="mm", bufs=2,
                                           name=f"{which}ps{m}_{j}")
                        for kt in range(NKT):
                            nc.tensor.matmul(
                                q_ps,
                                wt[kt][:, 128 * m:128 * m + 128],
                                xb_t[kt],
                                start=(kt == 0),
                                stop=(kt == NKT - 1),
                            )
                        q_sb = wpool.tile([128, TQ], BF16, tag="qsb",
                                          name=f"{which}sb{m}_{j}")
                        nc.vector.tensor_copy(q_sb, q_ps)
                        sq = wpool.tile([128, TQ], BF16, tag="sq",
                                        name=f"{which}sq{m}_{j}")
                        nc.vector.tensor_mul(sq, q_sb, q_sb)
                        ss_ps = pspool.tile([128, TQ], F32, tag="ss", bufs=1,
                                            name=f"{which}ss{m}_{j}")
                        nc.tensor.matmul(ss_ps, ones, sq, start=True, stop=True)
                        lnt = wpool.tile([128, TQ], F32, tag="lnt",
                                         name=f"{which}ln{m}_{j}")
                        nc.scalar.activation(lnt, ss_ps, AF.Ln,
                                             scale=1.0 / D, bias=epst)
                        rms = wpool.tile([128, TQ], BF16, tag="rms",
                                         name=f"{which}rms{m}_{j}")
                        nc.scalar.activation(rms, lnt, AF.Exp, scale=-0.5)
                        sw_ps = pspool.tile([128, TQ], F32, tag="swp", bufs=1,
                                            name=f"{which}swp{m}_{j}")
                        nc.tensor.matmul(sw_ps, perm_sb, q_sb,
                                         start=True, stop=True)
                        t1 = wpool.tile([128, TQ], BF16, tag="t1",
                                        name=f"{which}t1{m}_{j}")
                        nc.vector.tensor_mul(t1, q_sb, cos_sb[:, tq0:tq0 + TQ])
                        t2 = wpool.tile([128, TQ], BF16, tag="t2",
                                        name=f"{which}t2{m}_{j}")
                        nc.vector.tensor_mul(t2, sw_ps, sin_sb[:, tq0:tq0 + TQ])
                        t3 = wpool.tile([128, TQ], BF16, tag="t3",
                                        name=f"{which}t3{m}_{j}")
                        nc.vector.tensor_add(t3, t1, t2)
                        if which == "q":
                            dst = qpool.tile([128, TQ], BF16, tag=f"qT{m}",
                                             name=f"qT{m}_{j}")
                            nc.vector.tensor_mul(dst, t3, rms)
                            qT.append(dst)
                        else:
                            nc.vector.tensor_mul(kT[m][:, tq0:tq0 + TQ], t3, rms)

                # ---- v projection + value-residual blend ----
                for tt in range(4):
                    v_ps = pspool.tile([128, LC], F32, tag="mm", bufs=2,
                                       name=f"vps{tt}_{j}")
                    for kt in range(NKT):
                        nc.tensor.matmul(
                            v_ps,
                            xb_t[kt][:, 128 * tt:128 * tt + 128],
                            wv_t[kt],
                            start=(kt == 0),
                            stop=(kt == NKT - 1),
                        )
                    v1t = wpool.tile([128, LC], BF16, tag="v1t",
                                     name=f"v1t{tt}_{j}")
                    nc.sync.dma_start(
                        v1t, v1l[tq0 + 128 * tt:tq0 + 128 * tt + 128, :])
                    nc.vector.tensor_add(vst[4 * j + tt], v_ps, v1t)

                # ---- attention per head ----
                attnT = []
                ntk = 4 * (j + 1)
                for h in range(4):
                    pv_ps = pspool.tile([128, TQ], F32, tag="pv", bufs=1,
                                        name=f"pv{h}_{j}")
                    se_ps = pspool.tile([128, TQ], F32, tag="se", bufs=1,
                                        name=f"se{h}_{j}")
                    for tk in range(ntk):
                        s_ps = pspool.tile([128, TQ], F32, tag="s", bufs=2,
                                           name=f"s{h}_{tk}_{j}")
                        nc.tensor.matmul(
                            s_ps,
                            kT[h][:, 128 * tk:128 * tk + 128],
                            qT[h],
                            start=True,
                            stop=True,
                        )
                        if tk >= 4 * j:  # diagonal tile: causal mask add
                            d_ = 128 * tk - tq0
                            s0 = 384 - d_
                            nc.vector.tensor_add(
                                s_ps, s_ps, mask_sb[:, s0:s0 + TQ])
                        e_t = epool.tile([128, TQ], BF16, tag="e",
                                         name=f"e{h}_{tk}_{j}")
                        nc.scalar.activation(e_t, s_ps, AF.Exp, scale=SCALE)
                        nc.tensor.matmul(
                            pv_ps,
                            vst[tk][:, 128 * h:128 * h + 128],
                            e_t,
                            start=(tk == 0),
                            stop=(tk == ntk - 1),
                        )
                        nc.tensor.matmul(
                            se_ps, ones, e_t,
                            start=(tk == 0), stop=(tk == ntk - 1),
                        )
                    lnse = wpool.tile([128, TQ], F32, tag="lnse",
                                      name=f"lnse{h}_{j}")
                    nc.scalar.activation(lnse, se_ps, AF.Ln)
                    rec = wpool.tile([128, TQ], BF16, tag="rec",
                                     name=f"rec{h}_{j}")
                    nc.scalar.activation(rec, lnse, AF.Exp, scale=-1.0)
                    at = apool.tile([128, TQ], BF16, tag=f"attn{h}",
                                    name=f"attn{h}_{j}")
                    nc.vector.tensor_mul(at, pv_ps, rec)
                    attnT.append(at)

                # ---- partial c_proj ----
                for co in range(16):
                    o_ps = pspool.tile([128, TQ], F32, tag="mm", bufs=2,
                                       name=f"ops{co}_{j}")
                    for m in range(4):
                        nc.tensor.matmul(
                            o_ps,
                            wp_t[m][:, 128 * co:128 * co + 128],
                            attnT[m],
                            start=(m == 0),
                            stop=(m == 3),
                        )
                    o_sb = opool.tile([128, TQ], F32, tag="osb",
                                      name=f"osb{co}_{j}")
                    nc.vector.tensor_copy(o_sb, o_ps)
                    nc.sync.dma_start(
                        outT[128 * co:128 * co + 128, tq0:tq0 + TQ], o_sb)

    nc.finalize()
    return nc


def _host_prep(inputs):
    """Build the 8 per-core input maps (all numpy)."""
    x = np.asarray(inputs["x"], np.float32)
    v1 = np.asarray(inputs["v1"], np.float32)
    x_q = np.asarray(inputs["x_q"], np.float32)
    x_k = np.asarray(inputs["x_k"], np.float32)
    x_v = np.asarray(inputs["x_v"], np.float32)
    Wq = np.asarray(inputs["Wq"], np.float32)
    Wk = np.asarray(inputs["Wk"], np.float32)
    Wv = np.asarray(inputs["Wv"], np.float32)
    Wproj = np.asarray(inputs["Wproj"], np.float32)
    lamb = float(np.asarray(inputs["lamb"]))

    assert np.array_equal(x_q, x_k) and np.array_equal(x_q, x_v), (
        "kernel assumes shared token-shift mix vectors (x_q == x_k == x_v)"
    )

    # token-shift blend, then transpose per batch
    sh = np.concatenate([np.zeros((B, 1, C), np.float32), x[:, :-1]], axis=1)
    xb = x * (1.0 - x_q) + sh * x_q
    xbT = [np.ascontiguousarray(xb[b_].T).astype(_bf) for b_ in range(B)]

    # rope tables, duplicated halves; sin second half negated
    inv = 1.0 / (ROPE_THETA ** (np.arange(0, D, 2, dtype=np.float32) / D))
    fr = np.outer(np.arange(T, dtype=np.float32), inv)  # [T, 64]
    cosT = np.cos(fr).T.astype(np.float32)  # [64, T]
    sinT = np.sin(fr).T.astype(np.float32)
    cos2 = np.concatenate([cosT, cosT], axis=0).astype(_bf)
    sin2 = np.concatenate([sinT, -sinT], axis=0).astype(_bf)

    # causal mask master strip: M[p, g] = 0 if g >= p + 384 else MASK_NEG
    p = np.arange(128)[:, None]
    g = np.arange(896)[None, :]
    masks = np.where(g >= p + 384, 0.0, MASK_NEG).astype(np.float32)
    permm = np.roll(np.eye(128, dtype=np.float32), 64, axis=0).astype(_bf)

    in_maps = []
    for c in range(NCORES):
        b_ = c // 4
        g_ = c % 4
        L = slice(LC * g_, LC * g_ + LC)
        in_maps.append({
            "xbT": xbT[b_],
            "wq": np.ascontiguousarray(Wq[L].T).astype(_bf),
            "wk": np.ascontiguousarray(Wk[L].T).astype(_bf),
            "wv": np.ascontiguousarray(((1.0 - lamb) * Wv[L]).T).astype(_bf),
            "wp": np.ascontiguousarray(Wproj[:, L].T).astype(_bf),
            "v1l": (lamb * v1[b_][:, L]).astype(_bf),
            "cos2": cos2,
            "sin2": sin2,
            "masks": masks,
            "perm": permm,
        })
    return in_maps


def _run(in_maps, trace=False):
    from concourse.bass_utils import run_bass_kernel_spmd

    if "nc" not in _prog_cache:
        _prog_cache["nc"] = _build_program()
    return run_bass_kernel_spmd(
        _prog_cache["nc"], in_maps, core_ids=list(range(NCORES)), trace=trace
    )


def kernel(**inputs) -> np.ndarray:
    residual = np.asarray(inputs["residual"], np.float32)
    in_maps = _host_prep(inputs)
    res = _run(in_maps)
    out = np.empty((B, T, C), np.float32)
    for b_ in range(B):
        acc = res.results[4 * b_]["outT"].astype(np.float32)
        for g_ in range(1, 4):
            acc = acc + res.results[4 * b_ + g_]["outT"]
        out[b_] = residual[b_] + acc.T
    return out



# revision 4
# speedup vs baseline: 1.4090x; 1.4090x over previous
"""Causal self-attention (token-shift + QK-RMSNorm + RoPE + value-residual)
Trainium2 Bass kernel, sharded over 8 NeuronCores.

Sharding: core c handles batch b=c//4 and head-group g=c%4 (4 heads, 512
channels). Each core computes q/k/v projections for its channels, attention
for its heads, and a partial c_proj (its 512 input rows of Wproj). Host sums
the 4 partials per batch and adds the residual.

Projections (q/k/v/c_proj) run as fp8e4 DoubleRow matmuls (2 fp8 weights per
PE cell -> contraction 256 per pass). Weights are pre-scaled by 1024 on the
host: for q/k the scale cancels in the RMS norm; for v it is divided out in
the PSUM->SBUF blend; for c_proj the host divides the output by 32*1024
(attn tiles carry a further x32 folded into the softmax reciprocal so they
fit fp8 range). Attention score/PV matmuls stay bf16. Softmax needs no max
subtraction (|scores| <= sqrt(D) after RMS norm); the causal mask is a 0/1
bf16 multiply after exp; RoPE's half-swap is a SBUF->SBUF DMA partition
rotation (sign flip folded into the sin table host-side).
"""
import sys

sys.path.insert(0, "/opt/trn_rl_repo")

import numpy as np
import ml_dtypes

B, T, C, H, D = 2, 2048, 2048, 16, 128
NCORES = 8
LC = 512          # local channels per core (4 heads)
TQ = 512          # tq chunk size
NKS = C // 128    # 16 k-subtiles over the C contraction
NCHUNK = T // TQ  # 4
ROPE_THETA = 10000.0
EPS = float(np.finfo(np.float32).eps)
WSCALE = 1024.0   # fp8 weight pre-scale
ASCALE = 32.0     # fp8 attn-tile pre-scale (folded into softmax reciprocal)

_bf = ml_dtypes.bfloat16
_f8 = ml_dtypes.float8_e4m3fn

_prog_cache = {}


def _build_program():
    import concourse.bass as bass
    import concourse.mybir as mybir
    from concourse import bacc
    from concourse.tile import TileContext
    from concourse.alu_op_type import AluOpType

    AFt = mybir.ActivationFunctionType
    if not getattr(bacc, "_act_tables_pinned", False):
        _orig_gat = bacc.get_activation_tables

        def _pinned_gat(arch):
            tables = _orig_gat(arch)
            pinned = {AFt.Ln, AFt.Exp, AFt.Square}
            for name, fns in tables.items():
                if name != "natural_log_exp_and_others":
                    fns -= pinned
            return tables

        bacc.get_activation_tables = _pinned_gat
        bacc._act_tables_pinned = True

    F32 = mybir.dt.float32
    BF16 = mybir.dt.bfloat16
    FP8 = mybir.dt.float8e4
    DRm = mybir.MatmulPerfMode.DoubleRow
    AF = mybir.ActivationFunctionType

    nc = bacc.Bacc("TRN2", target_bir_lowering=False, debug=False)

    # xb8: [p, 16*j + s, t] = xb[C=128s+p, T=512j+t]  (chunk-major fp8)
    xb8 = nc.dram_tensor("xb8", [128, 64, TQ], FP8, kind="ExternalInput").ap()
    wq8 = nc.dram_tensor("wq8", [128, NKS, LC], FP8, kind="ExternalInput").ap()
    wk8 = nc.dram_tensor("wk8", [128, NKS, LC], FP8, kind="ExternalInput").ap()
    wv8 = nc.dram_tensor("wv8", [128, NKS, LC], FP8, kind="ExternalInput").ap()
    wp8 = nc.dram_tensor("wp8", [128, 4, C], FP8, kind="ExternalInput").ap()
    v1r = nc.dram_tensor("v1r", [128, 16, LC], BF16, kind="ExternalInput").ap()
    cos2 = nc.dram_tensor("cos2", [128, T], BF16, kind="ExternalInput").ap()
    sin2 = nc.dram_tensor("sin2", [128, T], BF16, kind="ExternalInput").ap()
    masks = nc.dram_tensor("masks", [128, 896], BF16, kind="ExternalInput").ap()
    outT = nc.dram_tensor("outT", [C, T], BF16, kind="ExternalOutput").ap()

    SCALE = 1.0 / float(np.sqrt(D))
    LN_AS = float(np.log(ASCALE))

    with TileContext(nc) as tc:
        with (
            tc.tile_pool(name="cpool", bufs=1) as cpool,
            tc.tile_pool(name="kvpool", bufs=1) as kvpool,
            tc.tile_pool(name="qpool", bufs=2) as qpool,
            tc.tile_pool(name="apool", bufs=2) as apool,
            tc.tile_pool(name="epool", bufs=6) as epool,
            tc.tile_pool(name="wpool", bufs=2) as wpool,
            tc.tile_pool(name="opool", bufs=3) as opool,
            tc.tile_pool(name="pspool", bufs=1, space="PSUM") as pspool,
        ):
            # ---- DMA loads, first-needed first ----
            wq_t = cpool.tile([128, NKS, LC], FP8, tag="wq", name="wq_t")
            nc.sync.dma_start(wq_t, wq8)
            wk_t = cpool.tile([128, NKS, LC], FP8, tag="wk", name="wk_t")
            nc.sync.dma_start(wk_t, wk8)
            xb_t = cpool.tile([128, 64, TQ], FP8, tag="xb", name="xb_t")
            nc.sync.dma_start(xb_t[:, 0:16, :], xb8[:, 0:16, :])
            cos_sb = cpool.tile([128, T], BF16, tag="cos", name="cos_sb")
            nc.sync.dma_start(cos_sb, cos2)
            sin_sb = cpool.tile([128, T], BF16, tag="sin", name="sin_sb")
            nc.sync.dma_start(sin_sb, sin2)
            mask_sb = cpool.tile([128, 896], BF16, tag="mask", name="mask_sb")
            nc.sync.dma_start(mask_sb, masks)
            wv_t = cpool.tile([128, NKS, LC], FP8, tag="wv", name="wv_t")
            nc.sync.dma_start(wv_t, wv8)
            v1_t = cpool.tile([128, 16, LC], BF16, tag="v1", name="v1_t")
            nc.sync.dma_start(v1_t, v1r)
            nc.sync.dma_start(xb_t[:, 16:32, :], xb8[:, 16:32, :])
            wp_t = cpool.tile([128, 4, C], FP8, tag="wp", name="wp_t")
            nc.sync.dma_start(wp_t, wp8)
            nc.sync.dma_start(xb_t[:, 32:48, :], xb8[:, 32:48, :])
            nc.sync.dma_start(xb_t[:, 48:64, :], xb8[:, 48:64, :])

            ones = cpool.tile([128, 128], BF16, tag="ones", name="ones")
            nc.vector.memset(ones, 1.0)
            epst = cpool.tile([128, 1], F32, tag="epst", name="epst")
            nc.vector.memset(epst, EPS)
            lnas = cpool.tile([128, 1], F32, tag="lnas", name="lnas")
            nc.vector.memset(lnas, LN_AS)

            # persistent K^T (per head) and V stores
            kT = [
                kvpool.tile([128, T], BF16, tag=f"kT{m}", name=f"kT{m}")
                for m in range(4)
            ]
            vst = [
                kvpool.tile([128, LC], BF16, tag=f"v{i}", name=f"v{i}")
                for i in range(T // 128)
            ]

            for j in range(NCHUNK):
                tq0 = TQ * j
                xc = xb_t[:, 16 * j:16 * j + 16, :]

                # ---- q/k projections + rmsnorm + rope ----
                qT = []
                for which, wt in (("q", wq_t), ("k", wk_t)):
                    for m in range(4):
                        q_ps = pspool.tile([128, TQ], F32, tag="mm", bufs=2,
                                           name=f"{which}ps{m}_{j}")
                        for i in range(NKS // 2):
                            nc.tensor.matmul(
                                q_ps,
                                wt[:, 2 * i:2 * i + 2, 128 * m:128 * m + 128],
                                xc[:, 2 * i:2 * i + 2, :],
                                start=(i == 0),
                                stop=(i == NKS // 2 - 1),
                                perf_mode=DRm,
                            )
                        q_sb = wpool.tile([128, TQ], BF16, tag="qsb",
                                          name=f"{which}sb{m}_{j}")
                        nc.vector.tensor_copy(q_sb, q_ps)
                        sq = wpool.tile([128, TQ], BF16, tag="sq",
                                        name=f"{which}sq{m}_{j}")
                        nc.vector.tensor_mul(sq, q_sb, q_sb)
                        ss_ps = pspool.tile([128, TQ], F32, tag="ss", bufs=2,
                                            name=f"{which}ss{m}_{j}")
                        nc.tensor.matmul(ss_ps, ones, sq, start=True, stop=True)
                        lnt = wpool.tile([128, TQ], F32, tag="lnt",
                                         name=f"{which}ln{m}_{j}")
                        nc.scalar.activation(lnt, ss_ps, AF.Ln,
                                             scale=1.0 / D, bias=epst)
                        rms = wpool.tile([128, TQ], BF16, tag="rms",
                                         name=f"{which}rms{m}_{j}")
                        nc.scalar.activation(rms, lnt, AF.Exp, scale=-0.5)
                        swp_sb = wpool.tile([128, TQ], BF16, tag="swp",
                                            name=f"{which}sw{m}_{j}")
                        nc.sync.dma_start(swp_sb[0:64, :], q_sb[64:128, :])
                        nc.sync.dma_start(swp_sb[64:128, :], q_sb[0:64, :])
                        t1 = wpool.tile([128, TQ], BF16, tag="t1",
                                        name=f"{which}t1{m}_{j}")
                        nc.vector.tensor_mul(t1, q_sb, cos_sb[:, tq0:tq0 + TQ])
                        t2 = wpool.tile([128, TQ], BF16, tag="t2",
                                        name=f"{which}t2{m}_{j}")
                        nc.vector.tensor_mul(t2, swp_sb, sin_sb[:, tq0:tq0 + TQ])
                        t3 = wpool.tile([128, TQ], BF16, tag="t3",
                                        name=f"{which}t3{m}_{j}")
                        nc.vector.tensor_add(t3, t1, t2)
                        if which == "q":
                            dst = qpool.tile([128, TQ], BF16, tag=f"qT{m}",
                                             name=f"qT{m}_{j}")
                            nc.vector.tensor_mul(dst, t3, rms)
                            qT.append(dst)
                        else:
                            nc.vector.tensor_mul(kT[m][:, tq0:tq0 + TQ], t3, rms)

                # ---- v projection + value-residual blend ----
                for tt in range(4):
                    v_ps = pspool.tile([128, LC], F32, tag="mm", bufs=2,
                                       name=f"vps{tt}_{j}")
                    for i in range(NKS // 2):
                        nc.tensor.matmul(
                            v_ps,
                            xc[:, 2 * i:2 * i + 2, 128 * tt:128 * tt + 128],
                            wv_t[:, 2 * i:2 * i + 2, :],
                            start=(i == 0),
                            stop=(i == NKS // 2 - 1),
                            perf_mode=DRm,
                        )
                    nc.vector.scalar_tensor_tensor(
                        vst[4 * j + tt], v_ps, 1.0 / WSCALE,
                        v1_t[:, 4 * j + tt, :],
                        AluOpType.mult, AluOpType.add,
                    )

                # ---- attention per head ----
                attn8 = apool.tile([128, 4, TQ], FP8, tag="attn8",
                                   name=f"attn8_{j}")
                ntk = 4 * (j + 1)
                for h in range(4):
                    pv_ps = pspool.tile([128, TQ], F32, tag="pv", bufs=1,
                                        name=f"pv{h}_{j}")
                    se_ps = pspool.tile([128, TQ], F32, tag="se", bufs=1,
                                        name=f"se{h}_{j}")
                    for tk in range(ntk):
                        s_ps = pspool.tile([128, TQ], F32, tag="s", bufs=2,
                                           name=f"s{h}_{tk}_{j}")
                        nc.tensor.matmul(
                            s_ps,
                            kT[h][:, 128 * tk:128 * tk + 128],
                            qT[h],
                            start=True,
                            stop=True,
                        )
                        e_t = epool.tile([128, TQ], BF16, tag="e",
                                         name=f"e{h}_{tk}_{j}")
                        nc.scalar.activation(e_t, s_ps, AF.Exp, scale=SCALE)
                        if tk >= 4 * j:  # diagonal tile: causal 0/1 mask
                            d_ = 128 * tk - tq0
                            s0 = 384 - d_
                            em = epool.tile([128, TQ], BF16, tag="e",
                                            name=f"em{h}_{tk}_{j}")
                            nc.vector.tensor_mul(
                                em, e_t, mask_sb[:, s0:s0 + TQ])
                            e_t = em
                        nc.tensor.matmul(
                            pv_ps,
                            vst[tk][:, 128 * h:128 * h + 128],
                            e_t,
                            start=(tk == 0),
                            stop=(tk == ntk - 1),
                        )
                        nc.tensor.matmul(
                            se_ps, ones, e_t,
                            start=(tk == 0), stop=(tk == ntk - 1),
                        )
                    lnse = wpool.tile([128, TQ], F32, tag="lnse",
                                      name=f"lnse{h}_{j}")
                    nc.scalar.activation(lnse, se_ps, AF.Ln)
                    rec = wpool.tile([128, TQ], BF16, tag="rec",
                                     name=f"rec{h}_{j}")
                    nc.scalar.activation(rec, lnse, AF.Exp, scale=-1.0,
                                         bias=lnas)
                    nc.vector.tensor_mul(attn8[:, h, :], pv_ps, rec)

                # ---- partial c_proj (output scaled by WSCALE*ASCALE) ----
                for co in range(16):
                    o_ps = pspool.tile([128, TQ], F32, tag="mm", bufs=2,
                                       name=f"ops{co}_{j}")
                    for i in range(2):
                        nc.tensor.matmul(
                            o_ps,
                            wp_t[:, 2 * i:2 * i + 2, 128 * co:128 * co + 128],
                            attn8[:, 2 * i:2 * i + 2, :],
                            start=(i == 0),
                            stop=(i == 1),
                            perf_mode=DRm,
                        )
                    o_sb = opool.tile([128, TQ], BF16, tag="osb",
                                      name=f"osb{co}_{j}")
                    nc.vector.tensor_copy(o_sb, o_ps)
                    nc.sync.dma_start(
                        outT[128 * co:128 * co + 128, tq0:tq0 + TQ], o_sb)

    nc.finalize()
    return nc


def _to8(a):
    return np.clip(a, -240.0, 240.0).astype(_f8)


def _ksub(a):
    """[C_like, M] -> [128, C_like//128, M] with k = 128*s + p."""
    k, m = a.shape
    return np.ascontiguousarray(a.reshape(k // 128, 128, m).transpose(1, 0, 2))


def _host_prep(inputs):
    """Build the 8 per-core input maps (all numpy)."""
    x = np.asarray(inputs["x"], np.float32)
    v1 = np.asarray(inputs["v1"], np.float32)
    x_q = np.asarray(inputs["x_q"], np.float32)
    x_k = np.asarray(inputs["x_k"], np.float32)
    x_v = np.asarray(inputs["x_v"], np.float32)
    Wq = np.asarray(inputs["Wq"], np.float32)
    Wk = np.asarray(inputs["Wk"], np.float32)
    Wv = np.asarray(inputs["Wv"], np.float32)
    Wproj = np.asarray(inputs["Wproj"], np.float32)
    lamb = float(np.asarray(inputs["lamb"]))

    assert np.array_equal(x_q, x_k) and np.array_equal(x_q, x_v), (
        "kernel assumes shared token-shift mix vectors (x_q == x_k == x_v)"
    )

    # token-shift blend, then transpose per batch
    sh = np.concatenate([np.zeros((B, 1, C), np.float32), x[:, :-1]], axis=1)
    xb = x * (1.0 - x_q) + sh * x_q
    # xb8[b][p, 16j+s, t] = xb[b].T[128s+p, 512j+t]
    xb8 = []
    for b_ in range(B):
        xt = xb[b_].T  # [C, T]
        a = xt.reshape(NKS, 128, NCHUNK, TQ).transpose(1, 2, 0, 3)
        xb8.append(_to8(a.reshape(128, NCHUNK * NKS, TQ)))

    # rope tables, duplicated halves; sin second half negated
    inv = 1.0 / (ROPE_THETA ** (np.arange(0, D, 2, dtype=np.float32) / D))
    fr = np.outer(np.arange(T, dtype=np.float32), inv)  # [T, 64]
    cosT = np.cos(fr).T.astype(np.float32)  # [64, T]
    sinT = np.sin(fr).T.astype(np.float32)
    cos2 = np.concatenate([cosT, cosT], axis=0).astype(_bf)
    sin2 = np.concatenate([sinT, -sinT], axis=0).astype(_bf)

    # causal mask master strip: M[p, g] = 1 if g >= p + 384 else 0
    p = np.arange(128)[:, None]
    g = np.arange(896)[None, :]
    masks = np.where(g >= p + 384, 1.0, 0.0).astype(_bf)

    in_maps = []
    for c in range(NCORES):
        b_ = c // 4
        g_ = c % 4
        L = slice(LC * g_, LC * g_ + LC)
        in_maps.append({
            "xb8": xb8[b_],
            "wq8": _to8(_ksub(WSCALE * Wq[L].T)),
            "wk8": _to8(_ksub(WSCALE * Wk[L].T)),
            "wv8": _to8(_ksub(WSCALE * (1.0 - lamb) * Wv[L].T)),
            "wp8": _to8(_ksub(WSCALE * Wproj[:, L].T)),
            "v1r": _ksub(lamb * v1[b_][:, L]).astype(_bf),
            "cos2": cos2,
            "sin2": sin2,
            "masks": masks,
        })
    return in_maps


def _run(in_maps, trace=False):
    from concourse.bass_utils import run_bass_kernel_spmd

    if "nc" not in _prog_cache:
        _prog_cache["nc"] = _build_program()
    return run_bass_kernel_spmd(
        _prog_cache["nc"], in_maps, core_ids=list(range(NCORES)), trace=trace
    )


def kernel(**inputs) -> np.ndarray:
    residual = np.asarray(inputs["residual"], np.float32)
    in_maps = _host_prep(inputs)
    res = _run(in_maps)
    out = np.empty((B, T, C), np.float32)
    descale = 1.0 / (WSCALE * ASCALE)
    for b_ in range(B):
        acc = res.results[4 * b_]["outT"].astype(np.float32)
        for g_ in range(1, 4):
            acc = acc + res.results[4 * b_ + g_]["outT"].astype(np.float32)
        out[b_] = residual[b_] + descale * acc.T
    return out


# revision 5
# speedup vs baseline: 1.6200x; 1.1497x over previous
"""Causal self-attention (token-shift + QK-RMSNorm + RoPE + value-residual)
Trainium2 Bass kernel, sharded over 8 NeuronCores.

Sharding: core c handles batch b=c//4 and head-group g=c%4 (4 heads, 512
channels). Each core computes q/k/v projections for its channels, attention
for its heads, and a partial c_proj (its 512 input rows of Wproj). Host sums
the 4 partials per batch and adds the residual.

Projections (q/k/v/c_proj) run as fp8e4 DoubleRow matmuls (2 fp8 weights per
PE cell -> contraction 256 per pass). Weights are pre-scaled by 1024 on the
host: for q/k the scale cancels in the RMS norm; for v it is divided out in
the PSUM->SBUF blend; for c_proj the host divides the output by 32*1024
(attn tiles carry a further x32 folded into the softmax reciprocal so they
fit fp8 range). Attention score/PV matmuls stay bf16. Softmax needs no max
subtraction (|scores| <= sqrt(D) after RMS norm); the causal mask reduces to
a 128-column staircase band per diagonal tile (prefix memset + 0/1 multiply
after exp); RoPE's half-swap is a SBUF->SBUF DMA partition rotation (sign
flip folded into the sin table host-side).

Emission is software-pipelined: chunk j+1's projection chains are
interleaved between chunk j's attention tiles so the PE never waits on the
exp->PV dependency chain.
"""
import sys

sys.path.insert(0, "/opt/trn_rl_repo")

import numpy as np
import ml_dtypes

B, T, C, H, D = 2, 2048, 2048, 16, 128
NCORES = 8
LC = 512          # local channels per core (4 heads)
TQ = 512          # tq chunk size
NKS = C // 128    # 16 k-subtiles over the C contraction
NCHUNK = T // TQ  # 4
ROPE_THETA = 10000.0
EPS = float(np.finfo(np.float32).eps)
WSCALE = 1024.0   # fp8 weight pre-scale
FP8_CPROJ = True  # c_proj in fp8 DoubleRow (else bf16)
ASCALE = 32.0 if FP8_CPROJ else 1.0

_bf = ml_dtypes.bfloat16
_f8 = ml_dtypes.float8_e4m3fn

_prog_cache = {}


def _build_program():
    import concourse.bass as bass
    import concourse.mybir as mybir
    from concourse import bacc
    from concourse.tile import TileContext
    from concourse.alu_op_type import AluOpType

    AFt = mybir.ActivationFunctionType
    if not getattr(bacc, "_act_tables_pinned", False):
        _orig_gat = bacc.get_activation_tables

        def _pinned_gat(arch):
            tables = _orig_gat(arch)
            pinned = {AFt.Ln, AFt.Exp, AFt.Square}
            for name, fns in tables.items():
                if name != "natural_log_exp_and_others":
                    fns -= pinned
            return tables

        bacc.get_activation_tables = _pinned_gat
        bacc._act_tables_pinned = True

    F32 = mybir.dt.float32
    BF16 = mybir.dt.bfloat16
    FP8 = mybir.dt.float8e4
    DRm = mybir.MatmulPerfMode.DoubleRow
    AF = mybir.ActivationFunctionType
    CPD = FP8 if FP8_CPROJ else BF16

    nc = bacc.Bacc("TRN2", target_bir_lowering=False, debug=False)

    # xb8: [p, 16*j + s, t] = xb[C=128s+p, T=512j+t]  (chunk-major fp8)
    xb8 = nc.dram_tensor("xb8", [128, 64, TQ], FP8, kind="ExternalInput").ap()
    wq8 = nc.dram_tensor("wq8", [128, NKS, LC], FP8, kind="ExternalInput").ap()
    wk8 = nc.dram_tensor("wk8", [128, NKS, LC], FP8, kind="ExternalInput").ap()
    wv8 = nc.dram_tensor("wv8", [128, NKS, LC], FP8, kind="ExternalInput").ap()
    wp8 = nc.dram_tensor("wp8", [128, 4, C], CPD, kind="ExternalInput").ap()
    v1r = nc.dram_tensor("v1r", [128, 16, LC], BF16, kind="ExternalInput").ap()
    cos2 = nc.dram_tensor("cos2", [128, T], BF16, kind="ExternalInput").ap()
    sin2 = nc.dram_tensor("sin2", [128, T], BF16, kind="ExternalInput").ap()
    maskb = nc.dram_tensor("maskb", [128, 128], BF16, kind="ExternalInput").ap()
    outT = nc.dram_tensor("outT", [C, T], BF16, kind="ExternalOutput").ap()

    SCALE = 1.0 / float(np.sqrt(D))
    LN_AS = float(np.log(ASCALE)) if ASCALE != 1.0 else 0.0

    def dma_split(dst, src, pieces, dim=1):
        n = dst.shape[dim]
        step = (n + pieces - 1) // pieces
        for o in range(0, n, step):
            sl = slice(o, min(o + step, n))
            if dim == 1:
                nc.sync.dma_start(dst[:, sl], src[:, sl])
            else:
                nc.sync.dma_start(dst[:, :, sl], src[:, :, sl])

    with TileContext(nc) as tc:
        with (
            tc.tile_pool(name="cpool", bufs=1) as cpool,
            tc.tile_pool(name="kvpool", bufs=1) as kvpool,
            tc.tile_pool(name="qpool", bufs=2) as qpool,
            tc.tile_pool(name="apool", bufs=2) as apool,
            tc.tile_pool(name="epool", bufs=6) as epool,
            tc.tile_pool(name="wpool", bufs=2) as wpool,
            tc.tile_pool(name="opool", bufs=3) as opool,
            tc.tile_pool(name="pspool", bufs=1, space="PSUM") as pspool,
        ):
            # ---- DMA loads, first-needed first, split across queues ----
            wq_t = cpool.tile([128, NKS, LC], FP8, tag="wq", name="wq_t")
            dma_split(wq_t, wq8, 4)
            wk_t = cpool.tile([128, NKS, LC], FP8, tag="wk", name="wk_t")
            dma_split(wk_t, wk8, 4)
            xb_t = cpool.tile([128, 4 * NKS, TQ], FP8, tag="xb", name="xb_t")
            dma_split(xb_t[:, 0:16, :], xb8[:, 0:16, :], 4)
            cos_sb = cpool.tile([128, T], BF16, tag="cos", name="cos_sb")
            dma_split(cos_sb, cos2, 2)
            sin_sb = cpool.tile([128, T], BF16, tag="sin", name="sin_sb")
            dma_split(sin_sb, sin2, 2)
            wv_t = cpool.tile([128, NKS, LC], FP8, tag="wv", name="wv_t")
            dma_split(wv_t, wv8, 4)
            v1_t = cpool.tile([128, 16, LC], BF16, tag="v1", name="v1_t")
            dma_split(v1_t, v1r, 8)
            mask_sb = cpool.tile([128, 128], BF16, tag="mask", name="mask_sb")
            nc.sync.dma_start(mask_sb, maskb)
            dma_split(xb_t[:, 16:32, :], xb8[:, 16:32, :], 4)
            wp_t = cpool.tile([128, 4, C], CPD, tag="wp", name="wp_t")
            dma_split(wp_t, wp8, 4)
            dma_split(xb_t[:, 32:48, :], xb8[:, 32:48, :], 4)
            dma_split(xb_t[:, 48:64, :], xb8[:, 48:64, :], 4)

            ones = cpool.tile([128, 128], BF16, tag="ones", name="ones")
            nc.vector.memset(ones, 1.0)
            epst = cpool.tile([128, 1], F32, tag="epst", name="epst")
            nc.vector.memset(epst, EPS)
            lnas = cpool.tile([128, 1], F32, tag="lnas", name="lnas")
            nc.vector.memset(lnas, LN_AS)

            # persistent K^T (per head) and V stores
            kT = [
                kvpool.tile([128, T], BF16, tag=f"kT{m}", name=f"kT{m}")
                for m in range(4)
            ]
            vst = [
                kvpool.tile([128, LC], BF16, tag=f"v{i}", name=f"v{i}")
                for i in range(T // 128)
            ]

            qT_all = {}

            def qk_chain(which, m, j):
                tq0 = TQ * j
                wt = wq_t if which == "q" else wk_t
                xc = xb_t[:, 16 * j:16 * j + 16, :]
                q_ps = pspool.tile([128, TQ], F32, tag="mm", bufs=2,
                                   name=f"{which}ps{m}_{j}")
                for i in range(NKS // 2):
                    nc.tensor.matmul(
                        q_ps,
                        wt[:, 2 * i:2 * i + 2, 128 * m:128 * m + 128],
                        xc[:, 2 * i:2 * i + 2, :],
                        start=(i == 0),
                        stop=(i == NKS // 2 - 1),
                        perf_mode=DRm,
                    )
                q_sb = wpool.tile([128, TQ], BF16, tag="qsb",
                                  name=f"{which}sb{m}_{j}")
                nc.vector.tensor_copy(q_sb, q_ps)
                sq = wpool.tile([128, TQ], BF16, tag="sq",
                                name=f"{which}sq{m}_{j}")
                nc.vector.tensor_mul(sq, q_sb, q_sb)
                ss_ps = pspool.tile([128, TQ], F32, tag="ss", bufs=2,
                                    name=f"{which}ss{m}_{j}")
                nc.tensor.matmul(ss_ps, ones, sq, start=True, stop=True)
                lnt = wpool.tile([128, TQ], F32, tag="lnt",
                                 name=f"{which}ln{m}_{j}")
                nc.scalar.activation(lnt, ss_ps, AF.Ln, scale=1.0 / D,
                                     bias=epst)
                rms = wpool.tile([128, TQ], BF16, tag="rms",
                                 name=f"{which}rms{m}_{j}")
                nc.scalar.activation(rms, lnt, AF.Exp, scale=-0.5)
                swp_sb = wpool.tile([128, TQ], BF16, tag="swp",
                                    name=f"{which}sw{m}_{j}")
                nc.sync.dma_start(swp_sb[0:64, :], q_sb[64:128, :])
                nc.sync.dma_start(swp_sb[64:128, :], q_sb[0:64, :])
                t1 = wpool.tile([128, TQ], BF16, tag="t1",
                                name=f"{which}t1{m}_{j}")
                nc.vector.tensor_mul(t1, q_sb, cos_sb[:, tq0:tq0 + TQ])
                t2 = wpool.tile([128, TQ], BF16, tag="t2",
                                name=f"{which}t2{m}_{j}")
                nc.vector.tensor_mul(t2, swp_sb, sin_sb[:, tq0:tq0 + TQ])
                t3 = wpool.tile([128, TQ], BF16, tag="t3",
                                name=f"{which}t3{m}_{j}")
                nc.vector.tensor_add(t3, t1, t2)
                if which == "q":
                    dst = qpool.tile([128, TQ], BF16, tag=f"qT{m}",
                                     name=f"qT{m}_{j}")
                    nc.vector.tensor_mul(dst, t3, rms)
                    qT_all[(m, j)] = dst
                else:
                    nc.vector.tensor_mul(kT[m][:, tq0:tq0 + TQ], t3, rms)

            def v_chain(tt, j):
                xc = xb_t[:, 16 * j:16 * j + 16, :]
                v_ps = pspool.tile([128, LC], F32, tag="mm", bufs=2,
                                   name=f"vps{tt}_{j}")
                for i in range(NKS // 2):
                    nc.tensor.matmul(
                        v_ps,
                        xc[:, 2 * i:2 * i + 2, 128 * tt:128 * tt + 128],
                        wv_t[:, 2 * i:2 * i + 2, :],
                        start=(i == 0),
                        stop=(i == NKS // 2 - 1),
                        perf_mode=DRm,
                    )
                nc.vector.scalar_tensor_tensor(
                    vst[4 * j + tt], v_ps, 1.0 / WSCALE,
                    v1_t[:, 4 * j + tt, :],
                    AluOpType.mult, AluOpType.add,
                )

            def proj_steps(j):
                steps = []
                for m in range(4):
                    steps.append(lambda m=m, j=j: qk_chain("q", m, j))
                    steps.append(lambda m=m, j=j: qk_chain("k", m, j))
                for tt in range(4):
                    steps.append(lambda tt=tt, j=j: v_chain(tt, j))
                return steps

            def attn_cproj(j, filler):
                tq0 = TQ * j
                ntk = 4 * (j + 1)
                units = 4 * ntk + 16
                fi = 0

                def drip(u):
                    # emit filler steps evenly across the units
                    nonlocal fi
                    want = ((u + 1) * len(filler)) // units
                    while fi < want:
                        filler[fi]()
                        fi += 1

                attn8 = apool.tile([128, 4, TQ], CPD, tag="attn8",
                                   name=f"attn8_{j}")
                u = 0
                for h in range(4):
                    pv_ps = pspool.tile([128, TQ], F32, tag="pv", bufs=1,
                                        name=f"pv{h}_{j}")
                    se_ps = pspool.tile([128, TQ], F32, tag="se", bufs=1,
                                        name=f"se{h}_{j}")
                    for tk in range(ntk):
                        s_ps = pspool.tile([128, TQ], F32, tag="s", bufs=2,
                                           name=f"s{h}_{tk}_{j}")
                        nc.tensor.matmul(
                            s_ps,
                            kT[h][:, 128 * tk:128 * tk + 128],
                            qT_all[(h, j)],
                            start=True,
                            stop=True,
                        )
                        e_t = epool.tile([128, TQ], BF16, tag="e",
                                         name=f"e{h}_{tk}_{j}")
                        if tk >= 4 * j:  # diagonal: staircase band mask
                            d_ = 128 * tk - tq0
                            if d_ > 0:
                                nc.vector.memset(e_t[:, 0:d_], 0.0)
                            nc.scalar.activation(e_t[:, d_:TQ],
                                                 s_ps[:, d_:TQ],
                                                 AF.Exp, scale=SCALE)
                            nc.vector.tensor_mul(
                                e_t[:, d_:d_ + 128],
                                e_t[:, d_:d_ + 128], mask_sb)
                        else:
                            nc.scalar.activation(e_t, s_ps, AF.Exp,
                                                 scale=SCALE)
                        nc.tensor.matmul(
                            pv_ps,
                            vst[tk][:, 128 * h:128 * h + 128],
                            e_t,
                            start=(tk == 0),
                            stop=(tk == ntk - 1),
                        )
                        nc.tensor.matmul(
                            se_ps, ones, e_t,
                            start=(tk == 0), stop=(tk == ntk - 1),
                        )
                        drip(u)
                        u += 1
                    lnse = wpool.tile([128, TQ], F32, tag="lnse",
                                      name=f"lnse{h}_{j}")
                    nc.scalar.activation(lnse, se_ps, AF.Ln)
                    rec = wpool.tile([128, TQ], BF16, tag="rec",
                                     name=f"rec{h}_{j}")
                    nc.scalar.activation(rec, lnse, AF.Exp, scale=-1.0,
                                         bias=lnas)
                    nc.vector.tensor_mul(attn8[:, h, :], pv_ps, rec)
                for co in range(16):
                    o_ps = pspool.tile([128, TQ], F32, tag="mm", bufs=2,
                                       name=f"ops{co}_{j}")
                    if FP8_CPROJ:
                        for i in range(2):
                            nc.tensor.matmul(
                                o_ps,
                                wp_t[:, 2 * i:2 * i + 2,
                                     128 * co:128 * co + 128],
                                attn8[:, 2 * i:2 * i + 2, :],
                                start=(i == 0),
                                stop=(i == 1),
                                perf_mode=DRm,
                            )
                    else:
                        for i in range(4):
                            nc.tensor.matmul(
                                o_ps,
                                wp_t[:, i, 128 * co:128 * co + 128],
                                attn8[:, i, :],
                                start=(i == 0),
                                stop=(i == 3),
                            )
                    o_sb = opool.tile([128, TQ], BF16, tag="osb",
                                      name=f"osb{co}_{j}")
                    nc.vector.tensor_copy(o_sb, o_ps)
                    nc.sync.dma_start(
                        outT[128 * co:128 * co + 128, tq0:tq0 + TQ], o_sb)
                    drip(u)
                    u += 1
                while fi < len(filler):
                    filler[fi]()
                    fi += 1

            # ---- software-pipelined emission ----
            for s in proj_steps(0):
                s()
            for j in range(NCHUNK):
                filler = proj_steps(j + 1) if j + 1 < NCHUNK else []
                attn_cproj(j, filler)

    nc.finalize()
    return nc


def _to8(a):
    return np.clip(a, -240.0, 240.0).astype(_f8)


def _ksub(a):
    """[C_like, M] -> [128, C_like//128, M] with k = 128*s + p."""
    k, m = a.shape
    return np.ascontiguousarray(a.reshape(k // 128, 128, m).transpose(1, 0, 2))


def _host_prep(inputs):
    """Build the 8 per-core input maps (all numpy)."""
    x = np.asarray(inputs["x"], np.float32)
    v1 = np.asarray(inputs["v1"], np.float32)
    x_q = np.asarray(inputs["x_q"], np.float32)
    x_k = np.asarray(inputs["x_k"], np.float32)
    x_v = np.asarray(inputs["x_v"], np.float32)
    Wq = np.asarray(inputs["Wq"], np.float32)
    Wk = np.asarray(inputs["Wk"], np.float32)
    Wv = np.asarray(inputs["Wv"], np.float32)
    Wproj = np.asarray(inputs["Wproj"], np.float32)
    lamb = float(np.asarray(inputs["lamb"]))

    assert np.array_equal(x_q, x_k) and np.array_equal(x_q, x_v), (
        "kernel assumes shared token-shift mix vectors (x_q == x_k == x_v)"
    )

    # token-shift blend, then transpose per batch
    sh = np.concatenate([np.zeros((B, 1, C), np.float32), x[:, :-1]], axis=1)
    xb = x * (1.0 - x_q) + sh * x_q
    # xb8[b][p, 16j+s, t] = xb[b].T[128s+p, 512j+t]
    xb8 = []
    for b_ in range(B):
        xt = xb[b_].T  # [C, T]
        a = xt.reshape(NKS, 128, NCHUNK, TQ).transpose(1, 2, 0, 3)
        xb8.append(_to8(a.reshape(128, NCHUNK * NKS, TQ)))

    # rope tables, duplicated halves; sin second half negated
    inv = 1.0 / (ROPE_THETA ** (np.arange(0, D, 2, dtype=np.float32) / D))
    fr = np.outer(np.arange(T, dtype=np.float32), inv)  # [T, 64]
    cosT = np.cos(fr).T.astype(np.float32)  # [64, T]
    sinT = np.sin(fr).T.astype(np.float32)
    cos2 = np.concatenate([cosT, cosT], axis=0).astype(_bf)
    sin2 = np.concatenate([sinT, -sinT], axis=0).astype(_bf)

    # staircase band mask: band[p, b] = 1 if b >= p else 0
    p = np.arange(128)[:, None]
    b = np.arange(128)[None, :]
    maskb = (b >= p).astype(np.float32).astype(_bf)

    if FP8_CPROJ:
        wp_prep = lambda a: _to8(_ksub(WSCALE * a))
    else:
        wp_prep = lambda a: _ksub(a).astype(_bf)

    in_maps = []
    for c in range(NCORES):
        b_ = c // 4
        g_ = c % 4
        L = slice(LC * g_, LC * g_ + LC)
        in_maps.append({
            "xb8": xb8[b_],
            "wq8": _to8(_ksub(WSCALE * Wq[L].T)),
            "wk8": _to8(_ksub(WSCALE * Wk[L].T)),
            "wv8": _to8(_ksub(WSCALE * (1.0 - lamb) * Wv[L].T)),
            "wp8": wp_prep(Wproj[:, L].T),
            "v1r": _ksub(lamb * v1[b_][:, L]).astype(_bf),
            "cos2": cos2,
            "sin2": sin2,
            "maskb": maskb,
        })
    return in_maps


def _run(in_maps, trace=False):
    from concourse.bass_utils import run_bass_kernel_spmd

    if "nc" not in _prog_cache:
        _prog_cache["nc"] = _build_program()
    return run_bass_kernel_spmd(
        _prog_cache["nc"], in_maps, core_ids=list(range(NCORES)), trace=trace
    )


def kernel(**inputs) -> np.ndarray:
    residual = np.asarray(inputs["residual"], np.float32)
    in_maps = _host_prep(inputs)
    res = _run(in_maps)
    out = np.empty((B, T, C), np.float32)
    descale = 1.0 / (WSCALE * ASCALE) if FP8_CPROJ else 1.0 / ASCALE
    for b_ in range(B):
        acc = res.results[4 * b_]["outT"].astype(np.float32)
        for g_ in range(1, 4):
            acc = acc + res.results[4 * b_ + g_]["outT"].astype(np.float32)
        out[b_] = residual[b_] + descale * acc.T
    return out
